# revision 1
# baseline (speedup 1.0000x reference)
"""AlphaFold-style gated MSA-row attention on 8 Trainium2 NeuronCores.

Shapes: q_data/kv_data [1,128,256,256], bias [1,128,8,256,256],
nonbatched_bias [1,8,256,256]; heads=8, c=32, out=256.

Strategy: pure data-parallel over b2 (128 rows -> 16 rows/core).
Per row, everything is computed in "transposed" activation layout so the
softmax matrix never needs an on-chip transpose:
  qT/kT/gT [hc, lq]  (hc = 8*32 = 256, two 128-partition chunks)
  v        [lk, hc]
  L^T      [lk, lq]  = (K Q^T) per head, + bias^T + nb^T via PE
                       identity-matmul accumulation into the same PSUM bank
  E^T      = exp(L^T)  (ACT, PSUM->SBUF)
  S_bc     = (2*ones)^T E^T  -> broadcast of 2*rowsum over 32 partitions/head
  waT      = v^T E^T (+head-packed via PSUM col offsets)
  m^T      = waT * (1 + tanh((g+bg)/2)) * recip(S_bc)   (2 fused DVE ops;
             0.5*(1+tanh(x/2)) == sigmoid(x), and the 0.5/S comes from the
             2*ones in S_bc)
  out      = m^T^T Wo^T + bo   (PE, + broadcast bo via fused DVE copy)

Matmuls run as float32r (full PE speed at N>=256, near-fp32 accuracy).
Host side only re-lays-out (swapaxes/tile) inputs - no arithmetic.
"""

import numpy as np

B1, B2, LQ, LK = 1, 128, 256, 256
QD = KVD = 256
H, C = 8, 32
HC = H * C          # 256
OD = 256
NCORES = 8
RPC = B2 // NCORES  # 16 rows per core

_CACHE = {}


def _build_nc():
    import concourse.bass as bass
    import concourse.bacc as bacc
    import concourse.mybir as mybir
    from concourse.tile import TileContext
    from concourse.bass import ts
    from concourse.tile_rust import add_dep_helper

    f32 = mybir.dt.float32
    f32r = mybir.dt.float32r
    bf16 = mybir.dt.bfloat16
    AF = mybir.ActivationFunctionType
    ALU = mybir.AluOpType

    nc = bacc.Bacc()

    # ---- DRAM parameters (per-core shard shapes) ----
    qdT_d = nc.declare_dram_parameter("qdT", [RPC // 2, 2, 128, 2, LQ], f32, isOutput=False)
    kvdT_d = nc.declare_dram_parameter("kvdT", [RPC // 2, 2, 128, 2, LK], f32, isOutput=False)
    biasT_d = nc.declare_dram_parameter("biasT", [RPC, 2, 128, H, LQ], f32, isOutput=False)
    nbT_d = nc.declare_dram_parameter("nbT", [2, 128, H, LQ], f32, isOutput=False)
    wqT_d = nc.declare_dram_parameter("wqT", [QD, HC], f32, isOutput=False)
    wkT_d = nc.declare_dram_parameter("wkT", [KVD, HC], f32, isOutput=False)
    wvT_d = nc.declare_dram_parameter("wvT", [KVD, HC], f32, isOutput=False)
    wgT_d = nc.declare_dram_parameter("wgT", [QD, HC], f32, isOutput=False)
    woT_d = nc.declare_dram_parameter("woT", [HC, OD], f32, isOutput=False)
    bg_d = nc.declare_dram_parameter("bg", [HC], f32, isOutput=False)
    bob_d = nc.declare_dram_parameter("bo_bcast", [128, OD], f32, isOutput=False)
    ident_d = nc.declare_dram_parameter("ident", [128, 128], f32, isOutput=False)
    two32_d = nc.declare_dram_parameter("two32", [128, 32], bf16, isOutput=False)
    out_d = nc.declare_dram_parameter("out", [RPC, LQ, OD], f32, isOutput=True)

    def r(ap):
        return ap if ap.dtype == f32r else ap.bitcast(f32r)

    def chain(mms):
        for a, b in zip(mms, mms[1:]):
            add_dep_helper(b.ins, a.ins, sync=False, reason="psum bank group order")

    with TileContext(nc) as tc:
        with tc.tile_pool(name="const", bufs=1) as constp, \
             tc.tile_pool(name="io", bufs=2) as iop, \
             tc.tile_pool(name="act", bufs=2) as actp, \
             tc.tile_pool(name="ps", bufs=1, space="PSUM") as psp:

            # ---- one-time loads ----
            wq = constp.tile([128, 2, HC], f32r, name="wq")
            wk = constp.tile([128, 2, HC], f32r, name="wk")
            wv = constp.tile([128, 2, HC], f32r, name="wv")
            wg = constp.tile([128, 2, HC], f32r, name="wg")
            wo = constp.tile([128, 2, OD], f32r, name="wo")
            for t, d in ((wq, wqT_d), (wk, wkT_d), (wv, wvT_d), (wg, wgT_d)):
                nc.sync.dma_start(out=t, in_=d.rearrange("(c p) h -> p c h", p=128).bitcast(f32r))
            nbt = constp.tile([128, 2, H, LQ], f32r, name="nbt")
            ident = constp.tile([128, 128], f32r, name="ident")
            two32 = constp.tile([128, 32], bf16, name="two32")
            bob = constp.tile([128, OD], f32, name="bob")
            bg_sb = constp.tile([128, 2], f32, name="bg_sb")
            bgh = constp.tile([128, 2], f32, name="bgh")

            def load_consts():
                nc.sync.dma_start(
                    out=wo, in_=woT_d.rearrange("(c p) h -> p c h", p=128).bitcast(f32r))
                for c2 in range(2):
                    nc.sync.dma_start(
                        out=nbt[:, c2, :, :], in_=nbT_d[c2].bitcast(f32r))
                nc.sync.dma_start(out=ident, in_=ident_d[:, :].bitcast(f32r))
                nc.sync.dma_start(out=two32, in_=two32_d[:, :])
                nc.sync.dma_start(out=bob, in_=bob_d[:, :])
                nc.sync.dma_start(out=bg_sb, in_=bg_d.rearrange("(c p) -> p c", p=128))
                nc.scalar.mul(bgh, bg_sb, 0.5)

            # zero-padded per-head kT [c-band padded to 128, head, lk-chunk, lk]
            # ring of 4: two pairs (4 rows) in flight under software pipelining
            skp = []
            for i in range(4):
                t = constp.tile([128, H, 2, 128], f32r, name=f"skp{i}")
                nc.vector.memset(t.bitcast(f32), 0.0)
                skp.append(t)

            qscale = float(C) ** -0.5

            def emit_pair(sp):
                # ---- input DMAs (pair): contiguous 2KB runs per partition ----
                qdt = iop.tile([128, 2, 2, LQ], f32r, tag="qdt", name="qdt")
                kvdt = iop.tile([128, 2, 2, LK], f32r, tag="kvdt", name="kvdt")
                for c2 in range(2):
                    nc.sync.dma_start(out=qdt[:, c2, :, :], in_=qdT_d[sp, c2].bitcast(f32r))
                    nc.sync.dma_start(out=kvdt[:, c2, :, :], in_=kvdT_d[sp, c2].bitcast(f32r))
                # ---- paired projections q/k/g (N=512 over 2 rows) ----
                pq_j = [psp.tile([128, 2, LQ], f32, tag="proj", bufs=2, name=f"pq{j}") for j in range(2)]
                pk_j = [psp.tile([128, 2, LK], f32, tag="proj", bufs=2, name=f"pk{j}") for j in range(2)]
                pg_j = [psp.tile([128, 2, LQ], f32, tag="proj", bufs=2, name=f"pg{j}") for j in range(2)]
                for pX, w, rhs_t in ((pq_j, wq, qdt), (pk_j, wk, kvdt), (pg_j, wg, qdt)):
                    for j in range(2):
                        mms = []
                        for c2 in range(2):
                            mms.append(nc.tensor.matmul(
                                pX[j][:, :, :], r(w[:, c2, ts(j, 128)]), rhs_t[:, c2, :, :],
                                start=(c2 == 0), stop=(c2 == 1)))
                        chain(mms)

                bias_p = []
                for rr in range(2):
                    bias_sb = iop.tile([128, 2, H, LQ], f32r, tag="bias", bufs=3, name="bias_sb")
                    for c2 in range(2):
                        nc.sync.dma_start(
                            out=bias_sb[:, c2, :, :],
                            in_=biasT_d[2 * sp + rr, c2].bitcast(f32r))
                    bias_p.append(bias_sb)
                if sp == 0:
                    load_consts()

                sq = actp.tile([128, 2, 2, LQ], f32r, tag="sq", name="sq")
                sk = actp.tile([128, 2, 2, LK], f32, tag="sk", name="sk")
                tv = actp.tile([128, 2, 2, LQ], f32, tag="tv", name="tv")
                for j in range(2):
                    nc.vector.tensor_scalar_mul(sq[:, j, :, :], pq_j[j], qscale)
                    nc.vector.tensor_copy(sk[:, j, :, :], pk_j[j])
                    nc.scalar.activation(tv[:, j, :, :], pg_j[j], AF.Tanh,
                                         bias=bgh[:, ts(j, 1)], scale=0.5)

                sv_r, skp_r = [], []
                for rr in range(2):
                    s = 2 * sp + rr
                    # ---- v projection (per row) ----
                    pv = psp.tile([128, 2, HC], f32, tag="proj", bufs=2, name="pv")
                    mm_v = []
                    for j in range(2):
                        for c2 in range(2):
                            mm_v.append(nc.tensor.matmul(
                                pv[:, j, :], kvdt[:, c2, rr, ts(j, 128)], r(wv[:, c2, :]),
                                start=(c2 == 0), stop=(c2 == 1)))
                    chain(mm_v)
                    sv = actp.tile([128, 2, HC], bf16, tag="sv", bufs=4, name="sv")
                    nc.vector.tensor_copy(sv, pv)
                    sv_r.append(sv)
                    # ---- zero-padded per-head kT ----
                    skp_s = skp[s % 4]
                    for h in range(H):
                        nc.vector.tensor_copy(
                            skp_s[ts(h % 4, 32), h, :, :], sk[ts(h % 4, 32), h // 4, rr, :])
                    skp_r.append(skp_s)
                return dict(sp=sp, bias_p=bias_p, sq=sq, tv=tv, sv_r=sv_r, skp_r=skp_r)

            def emit_attn(st):
                sp = st["sp"]
                sq, tv = st["sq"], st["tv"]
                for rr in range(2):
                    s = 2 * sp + rr
                    bias_sb = st["bias_p"][rr]
                    sv = st["sv_r"][rr]
                    skp_s = st["skp_r"][rr]

                    # ---- logits^T + bias^T + nb^T, then exp -> E^T ----
                    et = actp.tile([128, 2, H, LQ], bf16, tag="et", name="et")
                    for c2 in range(2):
                        for pr in range(4):
                            pL = psp.tile([128, 2, LQ], f32, tag="L", bufs=3, name="pL")
                            mm_l = []
                            for i2 in range(2):
                                h = 2 * pr + i2
                                mm_l.append(nc.tensor.matmul(
                                    pL[:, i2, :], skp_s[:, h, c2, :], sq[:, h // 4, rr, :],
                                    start=(i2 == 0), stop=False))
                            mm_l.append(nc.tensor.matmul(
                                pL[:, :, :], r(ident), r(nbt[:, c2, ts(pr, 2), :]),
                                start=False, stop=False))
                            mm_l.append(nc.tensor.matmul(
                                pL[:, :, :], r(ident), r(bias_sb[:, c2, ts(pr, 2), :]),
                                start=False, stop=True))
                            chain(mm_l)
                            nc.scalar.activation(et[:, c2, ts(pr, 2), :], pL, AF.Exp)

                    # ---- S broadcast (2*rowsum) and waT = v^T E^T ----
                    sbc = psp.tile([128, 2, LQ], f32, tag="sbc", bufs=1, name="sbc")
                    av = psp.tile([128, 2, LQ], f32, tag="av", bufs=1, name="av")
                    mm_s, mm_a = [], []
                    for g in range(2):
                        for c2 in range(2):
                            for hh in range(4):
                                h = 4 * g + hh
                                mm_s.append(nc.tensor.matmul(
                                    sbc[ts(hh, 32), g, :], two32, et[:, c2, h, :],
                                    start=(c2 == 0), stop=(c2 == 1),
                                    tile_position=(0, 32 * hh), skip_group_check=True))
                                mm_a.append(nc.tensor.matmul(
                                    av[ts(hh, 32), g, :], sv[:, c2, ts(h, 32)], et[:, c2, h, :],
                                    start=(c2 == 0), stop=(c2 == 1),
                                    tile_position=(0, 32 * hh), skip_group_check=True))
                    chain(mm_s)
                    chain(mm_a)
                    rs = actp.tile([128, 2, LQ], f32, tag="rs", name="rs")
                    nc.vector.reciprocal_approx_fast(out=rs, in_=sbc)

                    # ---- gate merge: m^T = waT*(1+tanh)*rs ----
                    tmp = actp.tile([128, 2, LQ], f32, tag="tmp", name="tmp")
                    for g in range(2):
                        nc.vector.scalar_tensor_tensor(
                            out=tmp[:, g, :], in0=tv[:, g, rr, :], scalar=1.0, in1=av[:, g, :],
                            op0=ALU.add, op1=ALU.mult)
                    mt = actp.tile([128, 2, LQ], f32r, tag="mt", name="mt")
                    nc.vector.scalar_tensor_tensor(
                        out=mt, in0=tmp, scalar=1.0, in1=rs, op0=ALU.mult, op1=ALU.mult)

                    # ---- output projection + bo ----
                    po = psp.tile([128, 2, OD], f32, tag="po", bufs=1, name="po")
                    mm_o = []
                    for m in range(2):
                        for j in range(2):
                            mm_o.append(nc.tensor.matmul(po[:, m, :], r(mt[:, j, ts(m, 128)]), r(wo[:, j, :]),
                                                         start=(j == 0), stop=(j == 1)))
                    chain(mm_o)
                    ob = actp.tile([128, 2, OD], f32, tag="ob", name="ob")
                    for m in range(2):
                        nc.vector.scalar_tensor_tensor(
                            out=ob[:, m, :], in0=po[:, m, :], scalar=1.0, in1=bob,
                            op0=ALU.mult, op1=ALU.add)
                    nc.sync.dma_start(
                        out=out_d[s].rearrange("(m p) o -> p m o", p=128), in_=ob)

            prev = None
            for sp in range(RPC // 2):
                st = emit_pair(sp)
                if prev is not None:
                    emit_attn(prev)
                prev = st
            emit_attn(prev)

    nc.compile()
    return nc


def _prep_inputs(q_data, kv_data, bias, nonbatched_bias, Wq, Wk, Wv, Wg, bg, Wo, bo):
    """Host-side data marshalling only (slicing / transposition / tiling)."""
    import ml_dtypes
    c = np.ascontiguousarray
    f = np.float32
    # [b2, d, lq] -> [b2/2, 2(c2), 128, 2(row), lq]: paired, 2KB-contiguous rows
    def pair_layout(x):  # x [b2, lq, d]
        xt = np.swapaxes(x, 1, 2)                    # [b2, d, lq]
        xt = xt.reshape(B2 // 2, 2, 2, 128, xt.shape[-1])  # [sp, r, c2, p, l]
        return c(np.transpose(xt, (0, 2, 3, 1, 4))).astype(f, copy=False)
    qdT = pair_layout(q_data[0])
    kvdT = pair_layout(kv_data[0])
    # [b2, lk, h, lq] -> [b2, 2, 128, h, lq]: per-partition-contiguous transposed bias
    biasT = c(np.transpose(bias[0], (0, 3, 1, 2))).astype(f, copy=False)
    biasT = biasT.reshape(B2, 2, 128, H, LQ)
    nbT = c(np.transpose(nonbatched_bias[0], (2, 0, 1))).astype(f, copy=False)
    nbT = nbT.reshape(2, 128, H, LQ)
    wqT = c(Wq.T).astype(f, copy=False)
    wkT = c(Wk.T).astype(f, copy=False)
    wvT = c(Wv.T).astype(f, copy=False)
    wgT = c(Wg.T).astype(f, copy=False)
    woT = c(Wo.T).astype(f, copy=False)
    bo_bcast = c(np.tile(np.asarray(bo, f)[None, :], (128, 1)))
    ident = np.eye(128, dtype=f)
    two32 = np.full((128, 32), 2.0, ml_dtypes.bfloat16)
    in_maps = []
    for core in range(NCORES):
        sl = slice(core * RPC, (core + 1) * RPC)
        in_maps.append(dict(
            qdT=c(qdT[core * RPC // 2:(core + 1) * RPC // 2]),
            kvdT=c(kvdT[core * RPC // 2:(core + 1) * RPC // 2]), biasT=c(biasT[sl]), nbT=nbT,
            wqT=wqT, wkT=wkT, wvT=wvT, wgT=wgT, woT=woT,
            bg=np.asarray(bg, f), bo_bcast=bo_bcast, ident=ident, two32=two32,
        ))
    return in_maps


def kernel(q_data, kv_data, bias, nonbatched_bias, Wq, Wk, Wv, Wg, bg, Wo, bo,
           _trace=False):
    from concourse.bass_utils import run_bass_kernel_spmd

    if "nc" not in _CACHE:
        _CACHE["nc"] = _build_nc()
    nc = _CACHE["nc"]
    in_maps = _prep_inputs(q_data, kv_data, bias, nonbatched_bias,
                           Wq, Wk, Wv, Wg, bg, Wo, bo)
    res = run_bass_kernel_spmd(nc, in_maps, list(range(NCORES)), trace=_trace)
    out = np.concatenate([np.asarray(res.results[i]["out"]) for i in range(NCORES)],
                         axis=0)
    out = out.reshape(B1, B2, LQ, OD).astype(np.float32, copy=False)
    if _trace:
        _CACHE["last_result"] = res
    return out



# revision 7
# speedup vs baseline: 1.1903x; 1.1903x over previous
"""AlphaFold-style gated MSA-row attention on 8 Trainium2 NeuronCores.

Shapes: q_data/kv_data [1,128,256,256], bias [1,128,8,256,256],
nonbatched_bias [1,8,256,256]; heads=8, c=32, out=256.

Strategy: pure data-parallel over b2 (128 rows -> 16 rows/core).
Per row, everything is computed in "transposed" activation layout so the
softmax matrix never needs an on-chip transpose:
  qT/kT/gT [hc, lq]  (hc = 8*32 = 256, two 128-partition chunks)
  v        [lk, hc]
  L^T      [lk, lq]  = (K Q^T) per head via K=32 row-tiled matmuls
                       (4 heads run concurrently in distinct 32-row PE
                       bands), + (bias+nb)^T via one identity-matmul
                       accumulation into the same PSUM bank
  E^T      = exp(L^T)  (ACT, PSUM->SBUF, bf16)
  S_bc     = (2*ones)^T E^T  -> broadcast of 2*rowsum over 32 partitions/head
  waT      = v^T E^T (head-packed via PSUM col tiling)
  m^T      = waT * (1 + tanh((g+bg)/2)) * recip(S_bc)   (2 fused DVE ops;
             0.5*(1+tanh(x/2)) == sigmoid(x), and the 0.5/S comes from the
             2*ones in S_bc)
  out      = m^T^T Wo^T + bo   (PE, + bo via fused DVE op)

The data path is bf16 (fp32 PSUM accumulation); the host pre-adds
nonbatched_bias into bias and casts inputs to bf16, halving the dominant
HBM stream (bias is 2/3 of all DMA traffic).
"""

import numpy as np

B1, B2, LQ, LK = 1, 128, 256, 256
QD = KVD = 256
H, C = 8, 32
HC = H * C          # 256
OD = 256
NCORES = 8
RPC = B2 // NCORES  # 16 rows per core

_CACHE = {}


def _build_nc():
    import concourse.bass as bass
    import concourse.bacc as bacc
    import concourse.mybir as mybir
    from concourse.tile import TileContext
    from concourse.bass import ts
    from concourse.tile_rust import add_dep_helper

    f32 = mybir.dt.float32
    bf16 = mybir.dt.bfloat16
    AF = mybir.ActivationFunctionType
    ALU = mybir.AluOpType

    nc = bacc.Bacc()

    # ---- DRAM parameters (per-core shard shapes) ----
    qdT_d = nc.declare_dram_parameter("qdT", [RPC // 2, 2, 128, 2, LQ], bf16, isOutput=False)
    kvdT_d = nc.declare_dram_parameter("kvdT", [RPC // 2, 2, 128, 2, LK], bf16, isOutput=False)
    bsumT_d = nc.declare_dram_parameter("bsumT", [RPC, 2, 128, H, LQ], bf16, isOutput=False)
    wqT_d = nc.declare_dram_parameter("wqT", [QD, HC], bf16, isOutput=False)
    wkT_d = nc.declare_dram_parameter("wkT", [KVD, HC], bf16, isOutput=False)
    wvT_d = nc.declare_dram_parameter("wvT", [KVD, HC], bf16, isOutput=False)
    wgT_d = nc.declare_dram_parameter("wgT", [QD, HC], bf16, isOutput=False)
    woT_d = nc.declare_dram_parameter("woT", [HC, OD], bf16, isOutput=False)
    bg_d = nc.declare_dram_parameter("bg", [HC], f32, isOutput=False)
    bob_d = nc.declare_dram_parameter("bo_bcast", [128, 2, OD], f32, isOutput=False)
    ident_d = nc.declare_dram_parameter("ident", [128, 128], bf16, isOutput=False)
    two32_d = nc.declare_dram_parameter("two32", [128, 32], bf16, isOutput=False)
    out_d = nc.declare_dram_parameter("out", [RPC, LQ, OD], f32, isOutput=True)

    def chain(mms):
        for a, b in zip(mms, mms[1:]):
            add_dep_helper(b.ins, a.ins, sync=False, reason="psum bank group order")

    with TileContext(nc) as tc:
        with tc.tile_pool(name="const", bufs=1) as constp, \
             tc.tile_pool(name="io", bufs=2) as iop, \
             tc.tile_pool(name="act", bufs=2) as actp, \
             tc.tile_pool(name="ps", bufs=1, space="PSUM") as psp:

            # ---- one-time loads ----
            wq = constp.tile([128, 2, HC], bf16, name="wq")
            wk = constp.tile([128, 2, HC], bf16, name="wk")
            wv = constp.tile([128, 2, HC], bf16, name="wv")
            wg = constp.tile([128, 2, HC], bf16, name="wg")
            wo = constp.tile([128, 2, OD], bf16, name="wo")
            for t, d in ((wq, wqT_d), (wk, wkT_d), (wv, wvT_d), (wg, wgT_d)):
                nc.sync.dma_start(out=t, in_=d.rearrange("(c p) h -> p c h", p=128))
            ident = constp.tile([128, 128], bf16, name="ident")
            two32 = constp.tile([128, 32], bf16, name="two32")
            bob = constp.tile([128, 2, OD], f32, name="bob")
            bg_sb = constp.tile([128, 2], f32, name="bg_sb")
            bgh = constp.tile([128, 2], f32, name="bgh")

            def load_consts():
                nc.sync.dma_start(
                    out=wo, in_=woT_d.rearrange("(c p) h -> p c h", p=128))
                nc.sync.dma_start(out=ident, in_=ident_d[:, :])
                nc.sync.dma_start(out=two32, in_=two32_d[:, :])
                nc.sync.dma_start(out=bob, in_=bob_d[:, :, :])
                nc.sync.dma_start(out=bg_sb, in_=bg_d.rearrange("(c p) -> p c", p=128))
                nc.scalar.mul(bgh, bg_sb, 0.5)

            qscale = float(C) ** -0.5

            def emit_pair(sp):
                # ---- input DMAs (pair) ----
                qdt = iop.tile([128, 2, 2, LQ], bf16, tag="qdt", name="qdt")
                kvdt = iop.tile([128, 2, 2, LK], bf16, tag="kvdt", name="kvdt")
                nc.sync.dma_start(out=qdt, in_=qdT_d[sp].rearrange("c p r l -> p c r l"))
                nc.sync.dma_start(out=kvdt, in_=kvdT_d[sp].rearrange("c p r l -> p c r l"))
                # ---- paired projections q/k/g (N=512 over 2 rows) ----
                pq_j = [psp.tile([128, 2, LQ], f32, tag="proj", bufs=2, name=f"pq{j}") for j in range(2)]
                pk_j = [psp.tile([128, 2, LK], f32, tag="proj", bufs=2, name=f"pk{j}") for j in range(2)]
                pg_j = [psp.tile([128, 2, LQ], f32, tag="proj", bufs=2, name=f"pg{j}") for j in range(2)]
                for pX, w, rhs_t in ((pq_j, wq, qdt), (pk_j, wk, kvdt), (pg_j, wg, qdt)):
                    for j in range(2):
                        mms = []
                        for c2 in range(2):
                            mms.append(nc.tensor.matmul(
                                pX[j][:, :, :], w[:, c2, ts(j, 128)], rhs_t[:, c2, :, :],
                                start=(c2 == 0), stop=(c2 == 1)))
                        chain(mms)

                bias_p = []
                for rr in range(2):
                    bias_sb = iop.tile([128, 2, H, LQ], bf16, tag="bias", bufs=3, name="bias_sb")
                    nc.sync.dma_start(
                        out=bias_sb,
                        in_=bsumT_d[2 * sp + rr].rearrange("c p h l -> p c h l"))
                    bias_p.append(bias_sb)
                if sp == 0:
                    load_consts()

                sq = actp.tile([128, 2, 2, LQ], bf16, tag="sq", name="sq")
                sk = actp.tile([128, 2, 2, LK], bf16, tag="sk", name="sk")
                tv = actp.tile([128, 2, 2, LQ], f32, tag="tv", name="tv")
                for j in range(2):
                    nc.vector.tensor_scalar_mul(sq[:, j, :, :], pq_j[j], qscale)
                    nc.vector.tensor_copy(sk[:, j, :, :], pk_j[j])
                    nc.scalar.activation(tv[:, j, :, :], pg_j[j], AF.Tanh,
                                         bias=bgh[:, ts(j, 1)], scale=0.5)

                sv_r = []
                for rr in range(2):
                    # ---- v projection (per row) ----
                    pv = psp.tile([128, 2, HC], f32, tag="proj", bufs=2, name="pv")
                    mm_v = []
                    for j in range(2):
                        for c2 in range(2):
                            mm_v.append(nc.tensor.matmul(
                                pv[:, j, :], kvdt[:, c2, rr, ts(j, 128)], wv[:, c2, :],
                                start=(c2 == 0), stop=(c2 == 1)))
                    chain(mm_v)
                    sv = actp.tile([128, 2, HC], bf16, tag="sv", bufs=4, name="sv")
                    nc.vector.tensor_copy(sv, pv)
                    sv_r.append(sv)
                return dict(sp=sp, bias_p=bias_p, sq=sq, sk=sk, tv=tv, sv_r=sv_r)

            def emit_attn(st):
                sp = st["sp"]
                sq, sk, tv = st["sq"], st["sk"], st["tv"]
                for rr in range(2):
                    s = 2 * sp + rr
                    bias_sb = st["bias_p"][rr]
                    sv = st["sv_r"][rr]

                    # ---- logits^T + (bias+nb)^T, then exp -> E^T ----
                    # K=32 row-tiled matmuls. PSUM rule: concurrent row tiles
                    # must hit distinct banks, so each 32-row PE band's head
                    # pair (h, h+4: same band, j=0/1) shares one 1-bank tile
                    # (serialized within the band by hardware), and the two
                    # bands of a phase run concurrently into 2 banks. The
                    # bias / E^T head axis is interleaved [0,4,1,5,2,6,3,7]
                    # to keep those pairs contiguous.
                    et = actp.tile([128, 2, H, LQ], bf16, tag="et", name="et")
                    for c2 in range(2):
                        for bp in range(2):
                            pLs, mm_l = [], []
                            for bb in range(2):
                                b = 2 * bp + bb
                                pL = psp.tile([128, 2, LQ], f32, tag="L", bufs=4, name="pL")
                                pLs.append((b, pL))
                                for j in range(2):
                                    mm_l.append(nc.tensor.matmul(
                                        pL[:, j, :],
                                        sk[ts(b, 32), j, rr, ts(c2, 128)],
                                        sq[ts(b, 32), j, rr, :],
                                        start=(j == 0), stop=False,
                                        tile_position=(32 * b, 0),
                                        skip_group_check=True))
                            for b, pL in pLs:
                                mm_l.append(nc.tensor.matmul(
                                    pL, ident,
                                    bias_sb[:, c2, ts(b, 2), :],
                                    start=False, stop=True,
                                    skip_group_check=True))
                            chain(mm_l)
                            for b, pL in pLs:
                                nc.scalar.activation(et[:, c2, ts(b, 2), :], pL, AF.Exp)

                    # ---- S broadcast (2*rowsum) and waT = v^T E^T ----
                    sbc = psp.tile([128, 2, LQ], f32, tag="sbc", bufs=1, name="sbc")
                    av = psp.tile([128, 2, LQ], f32, tag="av", bufs=1, name="av")
                    mm_s, mm_a = [], []
                    for g in range(2):
                        for c2 in range(2):
                            for hh in range(4):
                                h = 4 * g + hh
                                sl = 2 * hh + g  # interleaved E^T head slot
                                mm_s.append(nc.tensor.matmul(
                                    sbc[ts(hh, 32), g, :], two32, et[:, c2, sl, :],
                                    start=(c2 == 0), stop=(c2 == 1),
                                    tile_position=(0, 32 * hh), skip_group_check=True))
                                mm_a.append(nc.tensor.matmul(
                                    av[ts(hh, 32), g, :], sv[:, c2, ts(h, 32)], et[:, c2, sl, :],
                                    start=(c2 == 0), stop=(c2 == 1),
                                    tile_position=(0, 32 * hh), skip_group_check=True))
                    chain(mm_s)
                    chain(mm_a)
                    rs = actp.tile([128, 2, LQ], f32, tag="rs", name="rs")
                    nc.vector.reciprocal_approx_fast(out=rs, in_=sbc)

                    # ---- gate merge: m^T = waT*(1+tanh)*rs ----
                    tmp = actp.tile([128, 2, LQ], f32, tag="tmp", name="tmp")
                    nc.vector.scalar_tensor_tensor(
                        out=tmp, in0=tv[:, :, rr, :], scalar=1.0, in1=av,
                        op0=ALU.add, op1=ALU.mult)
                    mt = actp.tile([128, 2, LQ], bf16, tag="mt", name="mt")
                    nc.vector.scalar_tensor_tensor(
                        out=mt, in0=tmp, scalar=1.0, in1=rs, op0=ALU.mult, op1=ALU.mult)

                    # ---- output projection + bo ----
                    po = psp.tile([128, 2, OD], f32, tag="proj", bufs=2, name="po")
                    mm_o = []
                    for m in range(2):
                        for j in range(2):
                            mm_o.append(nc.tensor.matmul(po[:, m, :], mt[:, j, ts(m, 128)], wo[:, j, :],
                                                         start=(j == 0), stop=(j == 1)))
                    chain(mm_o)
                    ob = actp.tile([128, 2, OD], f32, tag="ob", name="ob")
                    nc.vector.scalar_tensor_tensor(
                        out=ob, in0=po, scalar=1.0, in1=bob,
                        op0=ALU.mult, op1=ALU.add)
                    nc.sync.dma_start(
                        out=out_d[s].rearrange("(m p) o -> p m o", p=128), in_=ob)

            prev = None
            for sp in range(RPC // 2):
                st = emit_pair(sp)
                if prev is not None:
                    emit_attn(prev)
                prev = st
            emit_attn(prev)

    nc.compile()
    return nc


def _prep_inputs(q_data, kv_data, bias, nonbatched_bias, Wq, Wk, Wv, Wg, bg, Wo, bo):
    """Host-side data marshalling: slicing / transposition / bf16 rounding,
    plus folding the replicated nonbatched_bias into the per-row bias."""
    import ml_dtypes
    bf = ml_dtypes.bfloat16
    c = np.ascontiguousarray
    f = np.float32
    # [b2, d, lq] -> [b2/2, 2(c2), 128, 2(row), lq]: paired rows
    def pair_layout(x):  # x [b2, lq, d]
        xt = np.swapaxes(x, 1, 2)                    # [b2, d, lq]
        xt = xt.reshape(B2 // 2, 2, 2, 128, xt.shape[-1])  # [sp, r, c2, p, l]
        return c(np.transpose(xt, (0, 2, 3, 1, 4)).astype(bf, copy=False))
    qdT = pair_layout(q_data[0])
    kvdT = pair_layout(kv_data[0])
    # bias + nb -> transposed [b2, lk, h, lq] in bf16, heads interleaved
    # [0,4,1,5,2,6,3,7] so PE-band pairs (h, h+4) are contiguous
    hperm = [0, 4, 1, 5, 2, 6, 3, 7]
    nbT = np.transpose(nonbatched_bias[0], (2, 0, 1))          # [lk, h, lq]
    bsumT = np.transpose(bias[0], (0, 3, 1, 2)) + nbT[None]    # [b2, lk, h, lq]
    bsumT = bsumT[:, :, hperm, :]
    bsumT = c(bsumT.astype(bf, copy=False)).reshape(B2, 2, 128, H, LQ)
    wqT = c(Wq.T.astype(bf, copy=False))
    wkT = c(Wk.T.astype(bf, copy=False))
    wvT = c(Wv.T.astype(bf, copy=False))
    wgT = c(Wg.T.astype(bf, copy=False))
    woT = c(Wo.T.astype(bf, copy=False))
    bo_bcast = c(np.tile(np.asarray(bo, f)[None, None, :], (128, 2, 1)))
    ident = np.eye(128, dtype=bf)
    two32 = np.full((128, 32), 2.0, bf)
    in_maps = []
    for core in range(NCORES):
        sl = slice(core * RPC, (core + 1) * RPC)
        in_maps.append(dict(
            qdT=c(qdT[core * RPC // 2:(core + 1) * RPC // 2]),
            kvdT=c(kvdT[core * RPC // 2:(core + 1) * RPC // 2]),
            bsumT=c(bsumT[sl]),
            wqT=wqT, wkT=wkT, wvT=wvT, wgT=wgT, woT=woT,
            bg=np.asarray(bg, f), bo_bcast=bo_bcast, ident=ident, two32=two32,
        ))
    return in_maps


def kernel(q_data, kv_data, bias, nonbatched_bias, Wq, Wk, Wv, Wg, bg, Wo, bo,
           _trace=False):
    from concourse.bass_utils import run_bass_kernel_spmd

    if "nc" not in _CACHE:
        _CACHE["nc"] = _build_nc()
    nc = _CACHE["nc"]
    in_maps = _prep_inputs(q_data, kv_data, bias, nonbatched_bias,
                           Wq, Wk, Wv, Wg, bg, Wo, bo)
    res = run_bass_kernel_spmd(nc, in_maps, list(range(NCORES)), trace=_trace)
    out = np.concatenate([np.asarray(res.results[i]["out"]) for i in range(NCORES)],
                         axis=0)
    out = out.reshape(B1, B2, LQ, OD).astype(np.float32, copy=False)
    if _trace:
        _CACHE["last_result"] = res
    return out


# revision 13
# speedup vs baseline: 1.1929x; 1.0022x over previous
"""AlphaFold-style gated MSA-row attention on 8 Trainium2 NeuronCores.

Shapes: q_data/kv_data [1,128,256,256], bias [1,128,8,256,256],
nonbatched_bias [1,8,256,256]; heads=8, c=32, out=256.

Strategy: pure data-parallel over b2 (128 rows -> 16 rows/core).
Per row, everything is computed in "transposed" activation layout so the
softmax matrix never needs an on-chip transpose:
  qT/kT/gT [hc, lq]  (hc = 8*32 = 256, two 128-partition chunks)
  v        [lk, hc]
  L^T      [lk, lq]  = (K Q^T) per head via K=32 row-tiled matmuls
                       (4 heads run concurrently in distinct 32-row PE
                       bands), + (bias+nb)^T via one identity-matmul
                       accumulation into the same PSUM bank
  E^T      = exp(L^T)  (ACT, PSUM->SBUF, bf16)
  S_bc     = (2*ones)^T E^T  -> broadcast of 2*rowsum over 32 partitions/head
  waT      = v^T E^T (head-packed via PSUM col tiling)
  m^T      = waT * (1 + tanh((g+bg)/2)) * recip(S_bc)   (2 fused DVE ops;
             0.5*(1+tanh(x/2)) == sigmoid(x), and the 0.5/S comes from the
             2*ones in S_bc)
  out      = m^T^T Wo^T + bo   (PE, + bo via fused DVE op)

The data path is bf16 (fp32 PSUM accumulation); the host pre-adds
nonbatched_bias into bias and casts inputs to bf16, halving the dominant
HBM stream (bias is 2/3 of all DMA traffic).
"""

import numpy as np

B1, B2, LQ, LK = 1, 128, 256, 256
QD = KVD = 256
H, C = 8, 32
HC = H * C          # 256
OD = 256
NCORES = 8
RPC = B2 // NCORES  # 16 rows per core

_CACHE = {}


def _build_nc():
    import concourse.bass as bass
    import concourse.bacc as bacc
    import concourse.mybir as mybir
    from concourse.tile import TileContext
    from concourse.bass import ts
    from concourse.tile_rust import add_dep_helper

    f32 = mybir.dt.float32
    bf16 = mybir.dt.bfloat16
    AF = mybir.ActivationFunctionType
    ALU = mybir.AluOpType

    nc = bacc.Bacc()

    # ---- DRAM parameters (per-core shard shapes) ----
    qdT_d = nc.declare_dram_parameter("qdT", [RPC // 2, 2, 128, 2, LQ], bf16, isOutput=False)
    kvdT_d = nc.declare_dram_parameter("kvdT", [RPC // 2, 2, 128, 2, LK], bf16, isOutput=False)
    bsumT_d = nc.declare_dram_parameter("bsumT", [RPC, 2, 128, H, LQ], bf16, isOutput=False)
    wqT_d = nc.declare_dram_parameter("wqT", [QD, HC], bf16, isOutput=False)
    wkT_d = nc.declare_dram_parameter("wkT", [KVD, HC], bf16, isOutput=False)
    wvT_d = nc.declare_dram_parameter("wvT", [KVD, HC], bf16, isOutput=False)
    wgT_d = nc.declare_dram_parameter("wgT", [QD, HC], bf16, isOutput=False)
    woT_d = nc.declare_dram_parameter("woT", [HC, OD], bf16, isOutput=False)
    bg_d = nc.declare_dram_parameter("bg", [HC], f32, isOutput=False)
    bob_d = nc.declare_dram_parameter("bo_bcast", [128, 2, OD], f32, isOutput=False)
    ident_d = nc.declare_dram_parameter("ident", [128, 128], bf16, isOutput=False)
    two32_d = nc.declare_dram_parameter("two32", [128, 32], bf16, isOutput=False)
    out_d = nc.declare_dram_parameter("out", [RPC, LQ, OD], f32, isOutput=True)

    def chain(mms):
        for a, b in zip(mms, mms[1:]):
            add_dep_helper(b.ins, a.ins, sync=False, reason="psum bank group order")

    with TileContext(nc) as tc:
        with tc.tile_pool(name="const", bufs=1) as constp, \
             tc.tile_pool(name="io", bufs=2) as iop, \
             tc.tile_pool(name="act", bufs=2) as actp, \
             tc.tile_pool(name="ps", bufs=1, space="PSUM") as psp:

            # ---- one-time loads ----
            wq = constp.tile([128, 2, HC], bf16, name="wq")
            wk = constp.tile([128, 2, HC], bf16, name="wk")
            wv = constp.tile([128, 2, HC], bf16, name="wv")
            wg = constp.tile([128, 2, HC], bf16, name="wg")
            wo = constp.tile([128, 2, OD], bf16, name="wo")
            for t, d in ((wq, wqT_d), (wk, wkT_d), (wv, wvT_d), (wg, wgT_d)):
                nc.sync.dma_start(out=t, in_=d.rearrange("(c p) h -> p c h", p=128))
            ident = constp.tile([128, 128], bf16, name="ident")
            two32 = constp.tile([128, 32], bf16, name="two32")
            bob = constp.tile([128, 2, OD], f32, name="bob")
            bg_sb = constp.tile([128, 2], f32, name="bg_sb")
            bgh = constp.tile([128, 2], f32, name="bgh")

            def load_consts():
                nc.sync.dma_start(
                    out=wo, in_=woT_d.rearrange("(c p) h -> p c h", p=128))
                nc.sync.dma_start(out=ident, in_=ident_d[:, :])
                nc.sync.dma_start(out=two32, in_=two32_d[:, :])
                nc.sync.dma_start(out=bob, in_=bob_d[:, :, :])
                nc.sync.dma_start(out=bg_sb, in_=bg_d.rearrange("(c p) -> p c", p=128))
                nc.scalar.mul(bgh, bg_sb, 0.5)

            qscale = float(C) ** -0.5

            def emit_pair(sp):
                # ---- input DMAs (pair) ----
                qdt = iop.tile([128, 2, 2, LQ], bf16, tag="qdt", bufs=3, name="qdt")
                kvdt = iop.tile([128, 2, 2, LK], bf16, tag="kvdt", bufs=3, name="kvdt")
                nc.sync.dma_start(out=qdt, in_=qdT_d[sp].rearrange("c p r l -> p c r l"))
                nc.sync.dma_start(out=kvdt, in_=kvdT_d[sp].rearrange("c p r l -> p c r l"))
                # ---- paired projections q/k/g (N=512 over 2 rows) ----
                pq_j = [psp.tile([128, 2, LQ], f32, tag="proj", bufs=2, name=f"pq{j}") for j in range(2)]
                pk_j = [psp.tile([128, 2, LK], f32, tag="proj", bufs=2, name=f"pk{j}") for j in range(2)]
                pg_j = [psp.tile([128, 2, LQ], f32, tag="proj", bufs=2, name=f"pg{j}") for j in range(2)]
                for pX, w, rhs_t in ((pq_j, wq, qdt), (pk_j, wk, kvdt), (pg_j, wg, qdt)):
                    for j in range(2):
                        mms = []
                        for c2 in range(2):
                            mms.append(nc.tensor.matmul(
                                pX[j][:, :, :], w[:, c2, ts(j, 128)], rhs_t[:, c2, :, :],
                                start=(c2 == 0), stop=(c2 == 1)))
                        chain(mms)

                bias_p = []
                for rr in range(2):
                    bias_sb = iop.tile([128, 2, H, LQ], bf16, tag="bias", bufs=4, name="bias_sb")
                    nc.sync.dma_start(
                        out=bias_sb,
                        in_=bsumT_d[2 * sp + rr].rearrange("c p h l -> p c h l"))
                    bias_p.append(bias_sb)

                sq = actp.tile([128, 2, 2, LQ], bf16, tag="sq", name="sq")
                sk = actp.tile([128, 2, 2, LK], bf16, tag="sk", name="sk")
                tv = actp.tile([128, 2, 2, LQ], f32, tag="tv", name="tv")
                for j in range(2):
                    nc.vector.tensor_scalar_mul(sq[:, j, :, :], pq_j[j], qscale)
                    nc.vector.tensor_copy(sk[:, j, :, :], pk_j[j])
                    nc.scalar.activation(tv[:, j, :, :], pg_j[j], AF.Tanh,
                                         bias=bgh[:, ts(j, 1)], scale=0.5)

                sv_r = []
                for rr in range(2):
                    # ---- v projection (per row) ----
                    pv = psp.tile([128, 2, HC], f32, tag="proj", bufs=2, name="pv")
                    mm_v = []
                    for j in range(2):
                        for c2 in range(2):
                            mm_v.append(nc.tensor.matmul(
                                pv[:, j, :], kvdt[:, c2, rr, ts(j, 128)], wv[:, c2, :],
                                start=(c2 == 0), stop=(c2 == 1)))
                    chain(mm_v)
                    sv = actp.tile([128, 2, HC], bf16, tag="sv", bufs=4, name="sv")
                    nc.vector.tensor_copy(sv, pv)
                    sv_r.append(sv)
                return dict(sp=sp, bias_p=bias_p, sq=sq, sk=sk, tv=tv, sv_r=sv_r)

            def emit_attn(st):
                sp = st["sp"]
                sq, sk, tv = st["sq"], st["sk"], st["tv"]
                for rr in range(2):
                    s = 2 * sp + rr
                    bias_sb = st["bias_p"][rr]
                    sv = st["sv_r"][rr]

                    # ---- logits^T + (bias+nb)^T, then exp -> E^T ----
                    # K=32 row-tiled matmuls. PSUM rule: concurrent row tiles
                    # must hit distinct banks, so each 32-row PE band's head
                    # pair (h, h+4: same band, j=0/1) shares one 1-bank tile
                    # (serialized within the band by hardware), and the two
                    # bands of a phase run concurrently into 2 banks. The
                    # bias / E^T head axis is interleaved [0,4,1,5,2,6,3,7]
                    # to keep those pairs contiguous.
                    et = actp.tile([128, 2, H, LQ], bf16, tag="et", name="et")
                    for c2 in range(2):
                        for bp in range(2):
                            # one 2-bank tile per phase: band bb -> bank bb
                            pL = psp.tile([128, 2, 2, LQ], f32, tag="L", bufs=2, name="pL")
                            mm_l = []
                            for bb in range(2):
                                b = 2 * bp + bb
                                for j in range(2):
                                    mm_l.append(nc.tensor.matmul(
                                        pL[:, bb, j, :],
                                        sk[ts(b, 32), j, rr, ts(c2, 128)],
                                        sq[ts(b, 32), j, rr, :],
                                        start=(j == 0), stop=False,
                                        tile_position=(32 * b, 0),
                                        skip_group_check=True))
                            for bb in range(2):
                                b = 2 * bp + bb
                                mm_l.append(nc.tensor.matmul(
                                    pL[:, bb, :, :], ident,
                                    bias_sb[:, c2, ts(b, 2), :],
                                    start=False, stop=True,
                                    skip_group_check=True))
                            chain(mm_l)
                            nc.scalar.activation(et[:, c2, ts(bp, 4), :], pL, AF.Exp)

                    # ---- S broadcast (2*rowsum) and waT = v^T E^T ----
                    sbc = psp.tile([128, 2, LQ], f32, tag="sbc", bufs=1, name="sbc")
                    av = psp.tile([128, 2, LQ], f32, tag="av", bufs=1, name="av")
                    mm_s, mm_a = [], []
                    for g in range(2):
                        for c2 in range(2):
                            for hh in range(4):
                                h = 4 * g + hh
                                sl = 2 * hh + g  # interleaved E^T head slot
                                mm_s.append(nc.tensor.matmul(
                                    sbc[ts(hh, 32), g, :], two32, et[:, c2, sl, :],
                                    start=(c2 == 0), stop=(c2 == 1),
                                    tile_position=(0, 32 * hh), skip_group_check=True))
                                mm_a.append(nc.tensor.matmul(
                                    av[ts(hh, 32), g, :], sv[:, c2, ts(h, 32)], et[:, c2, sl, :],
                                    start=(c2 == 0), stop=(c2 == 1),
                                    tile_position=(0, 32 * hh), skip_group_check=True))
                    chain(mm_s)
                    chain(mm_a)
                    rs = actp.tile([128, 2, LQ], f32, tag="rs", name="rs")
                    nc.vector.reciprocal_approx_fast(out=rs, in_=sbc)

                    # ---- gate merge: m^T = waT*(1+tanh)*rs ----
                    tmp = actp.tile([128, 2, LQ], f32, tag="tmp", name="tmp")
                    nc.vector.scalar_tensor_tensor(
                        out=tmp, in0=tv[:, :, rr, :], scalar=1.0, in1=av,
                        op0=ALU.add, op1=ALU.mult)
                    mt = actp.tile([128, 2, LQ], bf16, tag="mt", name="mt")
                    nc.vector.scalar_tensor_tensor(
                        out=mt, in0=tmp, scalar=1.0, in1=rs, op0=ALU.mult, op1=ALU.mult)

                    # ---- output projection + bo ----
                    po = psp.tile([128, 2, OD], f32, tag="proj", bufs=2, name="po")
                    mm_o = []
                    for m in range(2):
                        for j in range(2):
                            mm_o.append(nc.tensor.matmul(po[:, m, :], mt[:, j, ts(m, 128)], wo[:, j, :],
                                                         start=(j == 0), stop=(j == 1)))
                    chain(mm_o)
                    ob = actp.tile([128, 2, OD], f32, tag="ob", name="ob")
                    nc.vector.scalar_tensor_tensor(
                        out=ob, in0=po, scalar=1.0, in1=bob,
                        op0=ALU.mult, op1=ALU.add)
                    nc.sync.dma_start(
                        out=out_d[s].rearrange("(m p) o -> p m o", p=128), in_=ob)

            load_consts()
            prev = None
            for sp in range(RPC // 2):
                st = emit_pair(sp)
                if prev is not None:
                    emit_attn(prev)
                prev = st
            emit_attn(prev)

    nc.compile()
    return nc


def _prep_inputs(q_data, kv_data, bias, nonbatched_bias, Wq, Wk, Wv, Wg, bg, Wo, bo):
    """Host-side data marshalling: slicing / transposition / bf16 rounding,
    plus folding the replicated nonbatched_bias into the per-row bias."""
    import ml_dtypes
    bf = ml_dtypes.bfloat16
    c = np.ascontiguousarray
    f = np.float32
    # [b2, d, lq] -> [b2/2, 2(c2), 128, 2(row), lq]: paired rows
    def pair_layout(x):  # x [b2, lq, d]
        xt = np.swapaxes(x, 1, 2)                    # [b2, d, lq]
        xt = xt.reshape(B2 // 2, 2, 2, 128, xt.shape[-1])  # [sp, r, c2, p, l]
        return c(np.transpose(xt, (0, 2, 3, 1, 4)).astype(bf, copy=False))
    qdT = pair_layout(q_data[0])
    kvdT = pair_layout(kv_data[0])
    # bias + nb -> transposed [b2, lk, h, lq] in bf16, heads interleaved
    # [0,4,1,5,2,6,3,7] so PE-band pairs (h, h+4) are contiguous
    hperm = [0, 4, 1, 5, 2, 6, 3, 7]
    nbT = np.transpose(nonbatched_bias[0], (2, 0, 1))          # [lk, h, lq]
    bsumT = np.transpose(bias[0], (0, 3, 1, 2)) + nbT[None]    # [b2, lk, h, lq]
    bsumT = bsumT[:, :, hperm, :]
    bsumT = c(bsumT.astype(bf, copy=False)).reshape(B2, 2, 128, H, LQ)
    wqT = c(Wq.T.astype(bf, copy=False))
    wkT = c(Wk.T.astype(bf, copy=False))
    wvT = c(Wv.T.astype(bf, copy=False))
    wgT = c(Wg.T.astype(bf, copy=False))
    woT = c(Wo.T.astype(bf, copy=False))
    bo_bcast = c(np.tile(np.asarray(bo, f)[None, None, :], (128, 2, 1)))
    ident = np.eye(128, dtype=bf)
    two32 = np.full((128, 32), 2.0, bf)
    in_maps = []
    for core in range(NCORES):
        sl = slice(core * RPC, (core + 1) * RPC)
        in_maps.append(dict(
            qdT=c(qdT[core * RPC // 2:(core + 1) * RPC // 2]),
            kvdT=c(kvdT[core * RPC // 2:(core + 1) * RPC // 2]),
            bsumT=c(bsumT[sl]),
            wqT=wqT, wkT=wkT, wvT=wvT, wgT=wgT, woT=woT,
            bg=np.asarray(bg, f), bo_bcast=bo_bcast, ident=ident, two32=two32,
        ))
    return in_maps


def kernel(q_data, kv_data, bias, nonbatched_bias, Wq, Wk, Wv, Wg, bg, Wo, bo,
           _trace=False):
    from concourse.bass_utils import run_bass_kernel_spmd

    if "nc" not in _CACHE:
        _CACHE["nc"] = _build_nc()
    nc = _CACHE["nc"]
    in_maps = _prep_inputs(q_data, kv_data, bias, nonbatched_bias,
                           Wq, Wk, Wv, Wg, bg, Wo, bo)
    res = run_bass_kernel_spmd(nc, in_maps, list(range(NCORES)), trace=_trace)
    out = np.concatenate([np.asarray(res.results[i]["out"]) for i in range(NCORES)],
                         axis=0)
    out = out.reshape(B1, B2, LQ, OD).astype(np.float32, copy=False)
    if _trace:
        _CACHE["last_result"] = res
    return out


# revision 18
# speedup vs baseline: 1.1970x; 1.0034x over previous
"""AlphaFold-style gated MSA-row attention on 8 Trainium2 NeuronCores.

Shapes: q_data/kv_data [1,128,256,256], bias [1,128,8,256,256],
nonbatched_bias [1,8,256,256]; heads=8, c=32, out=256.

Strategy: pure data-parallel over b2 (128 rows -> 16 rows/core).
Per row, everything is computed in "transposed" activation layout so the
softmax matrix never needs an on-chip transpose:
  qT/kT/gT [hc, lq]  (hc = 8*32 = 256, two 128-partition chunks)
  v        [lk, hc]
  L^T      [lk, lq]  = (K Q^T) per head via K=32 row-tiled matmuls
                       (4 heads run concurrently in distinct 32-row PE
                       bands), + (bias+nb)^T via one identity-matmul
                       accumulation into the same PSUM bank
  E^T      = exp(L^T)  (ACT, PSUM->SBUF, bf16)
  S_bc     = (2*ones)^T E^T  -> broadcast of 2*rowsum over 32 partitions/head
  waT      = v^T E^T (head-packed via PSUM col tiling)
  m^T      = waT * (1 + tanh((g+bg)/2)) * recip(S_bc)   (2 fused DVE ops;
             0.5*(1+tanh(x/2)) == sigmoid(x), and the 0.5/S comes from the
             2*ones in S_bc)
  out      = m^T^T Wo^T + bo   (PE, + bo via fused DVE op)

The data path is bf16 (fp32 PSUM accumulation); the host pre-adds
nonbatched_bias into bias and casts inputs to bf16, halving the dominant
HBM stream (bias is 2/3 of all DMA traffic).
"""

import numpy as np

B1, B2, LQ, LK = 1, 128, 256, 256
QD = KVD = 256
H, C = 8, 32
HC = H * C          # 256
OD = 256
NCORES = 8
RPC = B2 // NCORES  # 16 rows per core

_CACHE = {}


def _build_nc():
    import concourse.bass as bass
    import concourse.bacc as bacc
    import concourse.mybir as mybir
    from concourse.tile import TileContext
    from concourse.bass import ts
    from concourse.tile_rust import add_dep_helper

    f32 = mybir.dt.float32
    bf16 = mybir.dt.bfloat16
    AF = mybir.ActivationFunctionType
    ALU = mybir.AluOpType

    nc = bacc.Bacc()

    # ---- DRAM parameters (per-core shard shapes) ----
    qdT_d = nc.declare_dram_parameter("qdT", [RPC // 2, 2, 128, 2, LQ], bf16, isOutput=False)
    kvdT_d = nc.declare_dram_parameter("kvdT", [RPC // 2, 2, 128, 2, LK], bf16, isOutput=False)
    bsumT_d = nc.declare_dram_parameter("bsumT", [RPC, 2, 128, H, LQ], bf16, isOutput=False)
    wqT_d = nc.declare_dram_parameter("wqT", [QD, HC], bf16, isOutput=False)
    wkT_d = nc.declare_dram_parameter("wkT", [KVD, HC], bf16, isOutput=False)
    wvT_d = nc.declare_dram_parameter("wvT", [KVD, HC], bf16, isOutput=False)
    wgT_d = nc.declare_dram_parameter("wgT", [QD, HC], bf16, isOutput=False)
    woT_d = nc.declare_dram_parameter("woT", [HC, OD], bf16, isOutput=False)
    bg_d = nc.declare_dram_parameter("bg", [HC], f32, isOutput=False)
    bob_d = nc.declare_dram_parameter("bo_bcast", [128, 2, OD], bf16, isOutput=False)
    ident_d = nc.declare_dram_parameter("ident", [128, 128], bf16, isOutput=False)
    two32_d = nc.declare_dram_parameter("two32", [128, 32], bf16, isOutput=False)
    out_d = nc.declare_dram_parameter("out", [RPC, LQ, OD], f32, isOutput=True)

    def chain(mms):
        for a, b in zip(mms, mms[1:]):
            add_dep_helper(b.ins, a.ins, sync=False, reason="psum bank group order")

    with TileContext(nc) as tc:
        with tc.tile_pool(name="const", bufs=1) as constp, \
             tc.tile_pool(name="io", bufs=2) as iop, \
             tc.tile_pool(name="act", bufs=2) as actp, \
             tc.tile_pool(name="ps", bufs=1, space="PSUM") as psp:

            # ---- one-time loads ----
            wq = constp.tile([128, 2, HC], bf16, name="wq")
            wk = constp.tile([128, 2, HC], bf16, name="wk")
            wv = constp.tile([128, 2, HC], bf16, name="wv")
            wg = constp.tile([128, 2, HC], bf16, name="wg")
            wo = constp.tile([128, 2, OD], bf16, name="wo")
            ident = constp.tile([128, 128], bf16, name="ident")
            two32 = constp.tile([128, 32], bf16, name="two32")
            bob = constp.tile([128, 2, OD], bf16, name="bob")
            bg_sb = constp.tile([128, 2], f32, name="bg_sb")
            bgh = constp.tile([128, 2], f32, name="bgh")

            def load_proj_weights():
                for t, d in ((wq, wqT_d), (wk, wkT_d), (wv, wvT_d), (wg, wgT_d)):
                    nc.sync.dma_start(out=t, in_=d.rearrange("(c p) h -> p c h", p=128))
                nc.sync.dma_start(out=bg_sb, in_=bg_d.rearrange("(c p) -> p c", p=128))
                nc.scalar.mul(bgh, bg_sb, 0.5)

            def load_consts():
                nc.sync.dma_start(out=ident, in_=ident_d[:, :])
                nc.sync.dma_start(
                    out=wo, in_=woT_d.rearrange("(c p) h -> p c h", p=128))
                nc.sync.dma_start(out=two32, in_=two32_d[:, :])
                nc.sync.dma_start(out=bob, in_=bob_d[:, :, :])

            qscale = float(C) ** -0.5

            def emit_pair(sp):
                # ---- input DMAs (pair) ----
                qdt = iop.tile([128, 2, 2, LQ], bf16, tag="qdt", bufs=3, name="qdt")
                kvdt = iop.tile([128, 2, 2, LK], bf16, tag="kvdt", bufs=3, name="kvdt")
                nc.sync.dma_start(out=qdt, in_=qdT_d[sp].rearrange("c p r l -> p c r l"))
                nc.sync.dma_start(out=kvdt, in_=kvdT_d[sp].rearrange("c p r l -> p c r l"))
                # ---- paired projections q/k/g (N=512 over 2 rows) ----
                pq_j = [psp.tile([128, 2, LQ], f32, tag="proj", bufs=2, name=f"pq{j}") for j in range(2)]
                pk_j = [psp.tile([128, 2, LK], f32, tag="proj", bufs=2, name=f"pk{j}") for j in range(2)]
                pg_j = [psp.tile([128, 2, LQ], f32, tag="proj", bufs=2, name=f"pg{j}") for j in range(2)]
                for pX, w, rhs_t in ((pq_j, wq, qdt), (pk_j, wk, kvdt), (pg_j, wg, qdt)):
                    for j in range(2):
                        mms = []
                        for c2 in range(2):
                            mms.append(nc.tensor.matmul(
                                pX[j][:, :, :], w[:, c2, ts(j, 128)], rhs_t[:, c2, :, :],
                                start=(c2 == 0), stop=(c2 == 1)))
                        chain(mms)

                bias_p = []
                for rr in range(2):
                    bias_sb = iop.tile([128, 2, H, LQ], bf16, tag="bias", bufs=4, name="bias_sb")
                    nc.sync.dma_start(
                        out=bias_sb,
                        in_=bsumT_d[2 * sp + rr].rearrange("c p h l -> p c h l"))
                    bias_p.append(bias_sb)

                sq = actp.tile([128, 2, 2, LQ], bf16, tag="sq", name="sq")
                sk = actp.tile([128, 2, 2, LK], bf16, tag="sk", name="sk")
                tv = actp.tile([128, 2, 2, LQ], f32, tag="tv", name="tv")
                for j in range(2):
                    nc.vector.tensor_scalar_mul(sq[:, j, :, :], pq_j[j], qscale)
                    nc.vector.tensor_copy(sk[:, j, :, :], pk_j[j])
                    nc.scalar.activation(tv[:, j, :, :], pg_j[j], AF.Tanh,
                                         bias=bgh[:, ts(j, 1)], scale=0.5)

                sv_r = []
                for rr in range(2):
                    # ---- v projection (per row) ----
                    pv = psp.tile([128, 2, HC], f32, tag="proj", bufs=2, name="pv")
                    mm_v = []
                    for j in range(2):
                        for c2 in range(2):
                            mm_v.append(nc.tensor.matmul(
                                pv[:, j, :], kvdt[:, c2, rr, ts(j, 128)], wv[:, c2, :],
                                start=(c2 == 0), stop=(c2 == 1)))
                    chain(mm_v)
                    sv = actp.tile([128, 2, HC], bf16, tag="sv", bufs=4, name="sv")
                    nc.vector.tensor_copy(sv, pv)
                    sv_r.append(sv)
                return dict(sp=sp, bias_p=bias_p, sq=sq, sk=sk, tv=tv, sv_r=sv_r)

            def emit_attn(st):
                sp = st["sp"]
                sq, sk, tv = st["sq"], st["sk"], st["tv"]
                for rr in range(2):
                    s = 2 * sp + rr
                    bias_sb = st["bias_p"][rr]
                    sv = st["sv_r"][rr]

                    # ---- logits^T + (bias+nb)^T, then exp -> E^T ----
                    # K=32 row-tiled matmuls. PSUM rule: concurrent row tiles
                    # must hit distinct banks, so each 32-row PE band's head
                    # pair (h, h+4: same band, j=0/1) shares one 1-bank tile
                    # (serialized within the band by hardware), and the two
                    # bands of a phase run concurrently into 2 banks. The
                    # bias / E^T head axis is interleaved [0,4,1,5,2,6,3,7]
                    # to keep those pairs contiguous.
                    et = actp.tile([128, 2, H, LQ], bf16, tag="et", name="et")
                    for c2 in range(2):
                        for bp in range(2):
                            # one 2-bank tile per phase: band bb -> bank bb
                            pL = psp.tile([128, 2, 2, LQ], f32, tag="L", bufs=2, name="pL")
                            mm_l = []
                            for bb in range(2):
                                b = 2 * bp + bb
                                for j in range(2):
                                    mm_l.append(nc.tensor.matmul(
                                        pL[:, bb, j, :],
                                        sk[ts(b, 32), j, rr, ts(c2, 128)],
                                        sq[ts(b, 32), j, rr, :],
                                        start=(j == 0), stop=False,
                                        tile_position=(32 * b, 0),
                                        skip_group_check=True))
                            for bb in range(2):
                                b = 2 * bp + bb
                                mm_l.append(nc.tensor.matmul(
                                    pL[:, bb, :, :], ident,
                                    bias_sb[:, c2, ts(b, 2), :],
                                    start=False, stop=True,
                                    skip_group_check=True))
                            chain(mm_l)
                            nc.scalar.activation(et[:, c2, ts(bp, 4), :], pL, AF.Exp)

                    # ---- S broadcast (2*rowsum) and waT = v^T E^T ----
                    sbc = psp.tile([128, 2, LQ], f32, tag="sbc", bufs=1, name="sbc")
                    av = psp.tile([128, 2, LQ], f32, tag="av", bufs=1, name="av")
                    mm_s, mm_a = [], []
                    for g in range(2):
                        for c2 in range(2):
                            for hh in range(4):
                                h = 4 * g + hh
                                sl = 2 * hh + g  # interleaved E^T head slot
                                mm_s.append(nc.tensor.matmul(
                                    sbc[ts(hh, 32), g, :], two32, et[:, c2, sl, :],
                                    start=(c2 == 0), stop=(c2 == 1),
                                    tile_position=(0, 32 * hh), skip_group_check=True))
                                mm_a.append(nc.tensor.matmul(
                                    av[ts(hh, 32), g, :], sv[:, c2, ts(h, 32)], et[:, c2, sl, :],
                                    start=(c2 == 0), stop=(c2 == 1),
                                    tile_position=(0, 32 * hh), skip_group_check=True))
                    chain(mm_s)
                    chain(mm_a)
                    rs = actp.tile([128, 2, LQ], f32, tag="rs", name="rs")
                    nc.vector.reciprocal_approx_fast(out=rs, in_=sbc)

                    # ---- gate merge: m^T = waT*(1+tanh)*rs ----
                    tmp = actp.tile([128, 2, LQ], f32, tag="tmp", name="tmp")
                    nc.vector.scalar_tensor_tensor(
                        out=tmp, in0=tv[:, :, rr, :], scalar=1.0, in1=av,
                        op0=ALU.add, op1=ALU.mult)
                    mt = actp.tile([128, 2, LQ], bf16, tag="mt", name="mt")
                    nc.vector.scalar_tensor_tensor(
                        out=mt, in0=tmp, scalar=1.0, in1=rs, op0=ALU.mult, op1=ALU.mult)

                    # ---- output projection + bo ----
                    po = psp.tile([128, 2, OD], f32, tag="proj", bufs=2, name="po")
                    mm_o = []
                    for m in range(2):
                        for j in range(2):
                            mm_o.append(nc.tensor.matmul(po[:, m, :], mt[:, j, ts(m, 128)], wo[:, j, :],
                                                         start=(j == 0), stop=(j == 1)))
                    chain(mm_o)
                    ob = actp.tile([128, 2, OD], f32, tag="ob", name="ob")
                    nc.vector.scalar_tensor_tensor(
                        out=ob, in0=po, scalar=1.0, in1=bob,
                        op0=ALU.mult, op1=ALU.add)
                    nc.sync.dma_start(
                        out=out_d[s].rearrange("(m p) o -> p m o", p=128), in_=ob)

            load_proj_weights()
            prev = None
            for sp in range(RPC // 2):
                st = emit_pair(sp)
                if sp == 0:
                    load_consts()
                if prev is not None:
                    emit_attn(prev)
                prev = st
            emit_attn(prev)

    nc.compile()
    return nc


def _prep_inputs(q_data, kv_data, bias, nonbatched_bias, Wq, Wk, Wv, Wg, bg, Wo, bo):
    """Host-side data marshalling: slicing / transposition / bf16 rounding,
    plus folding the replicated nonbatched_bias into the per-row bias."""
    import ml_dtypes
    bf = ml_dtypes.bfloat16
    c = np.ascontiguousarray
    f = np.float32
    # [b2, d, lq] -> [b2/2, 2(c2), 128, 2(row), lq]: paired rows
    def pair_layout(x):  # x [b2, lq, d]
        xt = np.swapaxes(x, 1, 2)                    # [b2, d, lq]
        xt = xt.reshape(B2 // 2, 2, 2, 128, xt.shape[-1])  # [sp, r, c2, p, l]
        return c(np.transpose(xt, (0, 2, 3, 1, 4)).astype(bf, copy=False))
    qdT = pair_layout(q_data[0])
    kvdT = pair_layout(kv_data[0])
    # bias + nb -> transposed [b2, lk, h, lq] in bf16, heads interleaved
    # [0,4,1,5,2,6,3,7] so PE-band pairs (h, h+4) are contiguous
    hperm = [0, 4, 1, 5, 2, 6, 3, 7]
    nbT = np.transpose(nonbatched_bias[0], (2, 0, 1))          # [lk, h, lq]
    bsumT = np.transpose(bias[0], (0, 3, 1, 2)) + nbT[None]    # [b2, lk, h, lq]
    bsumT = bsumT[:, :, hperm, :]
    bsumT = c(bsumT.astype(bf, copy=False)).reshape(B2, 2, 128, H, LQ)
    wqT = c(Wq.T.astype(bf, copy=False))
    wkT = c(Wk.T.astype(bf, copy=False))
    wvT = c(Wv.T.astype(bf, copy=False))
    wgT = c(Wg.T.astype(bf, copy=False))
    woT = c(Wo.T.astype(bf, copy=False))
    bo_bcast = c(np.tile(np.asarray(bo, f)[None, None, :], (128, 2, 1)).astype(bf))
    ident = np.eye(128, dtype=bf)
    two32 = np.full((128, 32), 2.0, bf)
    in_maps = []
    for core in range(NCORES):
        sl = slice(core * RPC, (core + 1) * RPC)
        in_maps.append(dict(
            qdT=c(qdT[core * RPC // 2:(core + 1) * RPC // 2]),
            kvdT=c(kvdT[core * RPC // 2:(core + 1) * RPC // 2]),
            bsumT=c(bsumT[sl]),
            wqT=wqT, wkT=wkT, wvT=wvT, wgT=wgT, woT=woT,
            bg=np.asarray(bg, f), bo_bcast=bo_bcast, ident=ident, two32=two32,
        ))
    return in_maps


def kernel(q_data, kv_data, bias, nonbatched_bias, Wq, Wk, Wv, Wg, bg, Wo, bo,
           _trace=False):
    from concourse.bass_utils import run_bass_kernel_spmd

    if "nc" not in _CACHE:
        _CACHE["nc"] = _build_nc()
    nc = _CACHE["nc"]
    in_maps = _prep_inputs(q_data, kv_data, bias, nonbatched_bias,
                           Wq, Wk, Wv, Wg, bg, Wo, bo)
    res = run_bass_kernel_spmd(nc, in_maps, list(range(NCORES)), trace=_trace)
    out = np.concatenate([np.asarray(res.results[i]["out"]) for i in range(NCORES)],
                         axis=0)
    out = out.reshape(B1, B2, LQ, OD).astype(np.float32, copy=False)
    if _trace:
        _CACHE["last_result"] = res
    return out
